# revision 9
# baseline (speedup 1.0000x reference)
"""Graph Wavelet Neural Network forward pass on 8 Trainium2 NeuronCores.

Computation: out = wavelets @ diag(filt) @ wavelets_inv @ features @ W
  N=8192, C_IN=256, C_OUT=128.

Strategy (memory-bound: the two [8192,8192] fp32 matrices dominate traffic):
  - Core j owns row-block jb = rows [j*1024, (j+1)*1024) of wavelets_inv
    (-> computes right rows jb) and column-block jb of wavelets
    (-> computes a full-shape partial of out; host sums the 8 partials).
    No device collectives needed.
  - All big operands are pre-transposed on the host so the contraction
    index lands on SBUF partitions and every DMA is contiguous:
      ft     = features.T                          [256, 8192]  (replicated)
      winv_t = (filt * wavelets_inv)[jb].T          [8192, 1024] (per-core)
      wav_t  = wavelets[:, jb].T                    [1024, 8192] (per-core)
    filt is folded into wavelets_inv rows on the host (free O(N^2)).
  - On device (core j):
      T    = features @ W          (PE form A, T natural k-tiles in SBUF)
      SR^T = sum_k T[k].T @ winv_t[k, :]   [128, 1024] psum, fp32r moving
      SR   = PE-transpose(SR^T)            8 tiles [128m, 128c]
      o^T  = sum_m SR[m].T @ wav_t[m, nch] [128, 512] psum per n-chunk
    Output per core: o^T partial [128, 8192] fp32.
  - fp32r (relaxed fp32) matmul: 1 cycle/row at moving free-dim >= 256 vs
    4 cycles/row for strict fp32 -> PE stays far below the DMA roofline.
"""

import os

import numpy as np

import concourse.bass as bass
import concourse.mybir as mybir
import concourse.tile as tile
from concourse import bacc
from concourse.bass_utils import run_bass_kernel_spmd
from concourse.masks import make_identity

N = 8192
C_IN = 256
C_OUT = 128
M = 8  # cores
B = N // M  # 1024 rows per core
KT = N // 128  # 64 contraction tiles
MT = B // 128  # 8 row tiles per core block
NCH = 512  # output free-dim chunk
F32 = mybir.dt.float32
F32R = mybir.dt.float32r

_cache = {}


def _build():
    nc = bacc.Bacc("TRN2", target_bir_lowering=False, debug=False)
    ft = nc.dram_tensor("ft", [C_IN, N], F32, kind="ExternalInput")
    wm = nc.dram_tensor("wm", [C_IN, C_OUT], F32, kind="ExternalInput")
    winv_t = nc.dram_tensor("winv_t", [N, B], F32R, kind="ExternalInput")
    wav_t = nc.dram_tensor("wav_t", [B, N], F32R, kind="ExternalInput")
    outp = nc.dram_tensor("outp", [C_OUT, N], F32, kind="ExternalOutput")

    with tile.TileContext(nc) as tc:
        with (
            tc.tile_pool(name="const", bufs=1) as cpool,
            tc.tile_pool(name="stream", bufs=4) as spool,
            tc.tile_pool(name="opool", bufs=3) as opool,
            tc.tile_pool(name="ps_small", bufs=2, space="PSUM") as ps_small,
            tc.tile_pool(name="ps_r", bufs=1, space="PSUM") as ps_r,
            tc.tile_pool(name="ps_o", bufs=2, space="PSUM") as ps_o,
        ):
            # --- constants / small inputs (ACT ring, decoupled from bulk) ---
            ident = cpool.tile([128, 128], F32, tag="ident")
            make_identity(nc, ident)
            wm_sb = cpool.tile([128, 2 * C_OUT], F32, tag="wm")
            for k2 in range(2):
                nc.scalar.dma_start(
                    out=wm_sb[:, k2 * C_OUT : (k2 + 1) * C_OUT],
                    in_=wm.ap()[k2 * 128 : (k2 + 1) * 128, :],
                )
            ft_sb = [
                cpool.tile([128, N], F32, tag=f"ft{k2}", name=f"ft_sb{k2}")
                for k2 in range(2)
            ]
            for k2 in range(2):
                nc.scalar.dma_start(out=ft_sb[k2], in_=ft.ap()[k2 * 128 : (k2 + 1) * 128, :])

            # --- stage A: T = F @ W, natural k-tiles, 4 per [128,512] group ---
            t_sb = [
                cpool.tile([128, 4 * 128], F32R, tag=f"T{g}", name=f"t_sb{g}")
                for g in range(KT // 4)
            ]
            for g in range(KT // 4):
                ps = ps_small.tile([128, 512], F32, tag="psA")
                for i in range(4):
                    n_tile = g * 4 + i
                    for k2 in range(2):
                        nc.tensor.matmul(
                            ps[:, i * 128 : (i + 1) * 128],
                            ft_sb[k2][:, n_tile * 128 : (n_tile + 1) * 128],
                            wm_sb[:, k2 * C_OUT : (k2 + 1) * C_OUT],
                            start=(k2 == 0),
                            stop=(k2 == 1),
                        )
                nc.vector.tensor_copy(t_sb[g], ps)

            # --- stage B: SR^T accumulation over 64 k-tiles ---
            ps_sr = ps_r.tile([128, B], F32, tag="psR")
            for k in range(KT):
                wi = spool.tile([128, B], F32R, tag="wi")
                nc.sync.dma_start(out=wi, in_=winv_t.ap()[k * 128 : (k + 1) * 128, :])
                lhs = t_sb[k // 4][:, (k % 4) * 128 : (k % 4 + 1) * 128]
                for h in range(2):
                    nc.tensor.matmul(
                        ps_sr[:, h * 512 : (h + 1) * 512],
                        lhs,
                        wi[:, h * 512 : (h + 1) * 512],
                        start=(k == 0),
                        stop=(k == KT - 1),
                    )

            srT = cpool.tile([128, B], F32, tag="srT")
            nc.vector.tensor_copy(srT, ps_sr)

            # --- stage C: SR tiles = transpose(SR^T) ---
            sr_sb = [
                cpool.tile([128, 128], F32R, tag=f"sr{mt}", name=f"sr_sb{mt}")
                for mt in range(MT)
            ]
            for mt in range(MT):
                pst = ps_small.tile([128, 128], F32, tag="psT")
                nc.tensor.transpose(pst, srT[:, mt * 128 : (mt + 1) * 128], ident)
                nc.vector.tensor_copy(sr_sb[mt], pst)

            # --- stage D: out^T partial chunks ---
            for ncch in range(N // NCH):
                ps_out = ps_o.tile([128, NCH], F32, tag="psO")
                for mt in range(MT):
                    wv = spool.tile([128, NCH], F32R, tag="wv")
                    nc.sync.dma_start(
                        out=wv,
                        in_=wav_t.ap()[
                            mt * 128 : (mt + 1) * 128, ncch * NCH : (ncch + 1) * NCH
                        ],
                    )
                    nc.tensor.matmul(
                        ps_out,
                        sr_sb[mt],
                        wv,
                        start=(mt == 0),
                        stop=(mt == MT - 1),
                    )
                ot = opool.tile([128, NCH], F32, tag="ot")
                nc.vector.tensor_copy(ot, ps_out)
                nc.scalar.dma_start(
                    out=outp.ap()[:, ncch * NCH : (ncch + 1) * NCH], in_=ot
                )
    nc.compile()
    return nc


def kernel(features, wavelets, wavelets_inv, weight_matrix, filt):
    os.environ.setdefault("BASS_NEVER_TRACE", "1")
    if "nc" not in _cache:
        _cache["nc"] = _build()
    nc = _cache["nc"]

    features = np.ascontiguousarray(features, dtype=np.float32)
    wavelets = np.ascontiguousarray(wavelets, dtype=np.float32)
    wavelets_inv = np.ascontiguousarray(wavelets_inv, dtype=np.float32)
    weight_matrix = np.ascontiguousarray(weight_matrix, dtype=np.float32)
    filt = np.ascontiguousarray(filt, dtype=np.float32)

    ft = np.ascontiguousarray(features.T)
    in_maps = []
    for j in range(M):
        jb = slice(j * B, (j + 1) * B)
        winv_t = np.ascontiguousarray((wavelets_inv[jb, :] * filt[jb, None]).T)
        wav_t = np.ascontiguousarray(wavelets[:, jb].T)
        in_maps.append({"ft": ft, "wm": weight_matrix, "winv_t": winv_t, "wav_t": wav_t})

    res = run_bass_kernel_spmd(nc, in_maps, core_ids=list(range(M)))
    acc = res.results[0]["outp"].astype(np.float64)
    for j in range(1, M):
        acc += res.results[j]["outp"]
    return np.ascontiguousarray(acc.T.astype(np.float32))


# revision 17
# speedup vs baseline: 1.2837x; 1.2837x over previous
"""Graph Wavelet Neural Network forward pass on 8 Trainium2 NeuronCores.

Computation: out = wavelets @ diag(filt) @ wavelets_inv @ features @ W
  N=8192, C_IN=256, C_OUT=128.

Strategy (memory-bound: the two [8192,8192] fp32 matrices dominate traffic):
  - Core j owns row-block jb = rows [j*1024, (j+1)*1024) of wavelets_inv
    (-> computes right rows jb) and column-block jb of wavelets
    (-> computes a full-shape partial of out; host sums the 8 partials).
    No device collectives needed.
  - All big operands are pre-transposed on the host so the contraction
    index lands on SBUF partitions and every DMA is contiguous:
      ft     = features.T                          [256, 8192]  (replicated)
      winv_t = (filt * wavelets_inv)[jb].T          [8192, 1024] (per-core)
      wav_t  = wavelets[:, jb].T                    [1024, 8192] (per-core)
    filt is folded into wavelets_inv rows on the host (free O(N^2)).
  - On device (core j):
      T    = features @ W          (PE form A, T natural k-tiles in SBUF)
      SR^T = sum_k T[k].T @ winv_t[k, :]   [128, 1024] psum, fp32r moving
      SR   = PE-transpose(SR^T)            8 tiles [128m, 128c]
      o^T  = sum_m SR[m].T @ wav_t[m, nch] [128, 512] psum per n-chunk
    Output per core: o^T partial [128, 8192] fp32.
  - fp32r (relaxed fp32) matmul: 1 cycle/row at moving free-dim >= 256 vs
    4 cycles/row for strict fp32 -> PE stays far below the DMA roofline.
"""

import os

import numpy as np

import concourse.bass as bass
import concourse.mybir as mybir
import concourse.tile as tile
from concourse import bacc
from concourse.bass_utils import run_bass_kernel_spmd
from concourse.masks import make_identity

N = 8192
C_IN = 256
C_OUT = 128
M = 8  # cores
B = N // M  # 1024 rows per core
KT = N // 128  # 64 contraction tiles
MT = B // 128  # 8 row tiles per core block
NCH = 1024  # output free-dim chunk
F32 = mybir.dt.float32
F32R = mybir.dt.float32r

_cache = {}


def _build():
    nc = bacc.Bacc("TRN2", target_bir_lowering=False, debug=False)
    ft = nc.dram_tensor("ft", [C_IN, N], F32R, kind="ExternalInput")
    wm = nc.dram_tensor("wm", [C_IN, C_OUT], F32R, kind="ExternalInput")
    winv_t = nc.dram_tensor("winv_t", [N, B], F32R, kind="ExternalInput")
    wav_t = nc.dram_tensor("wav_t", [B, N], F32R, kind="ExternalInput")
    outp = nc.dram_tensor("outp", [C_OUT, N], F32, kind="ExternalOutput")

    with tile.TileContext(nc) as tc:
        with (
            tc.tile_pool(name="const", bufs=1) as cpool,
            tc.tile_pool(name="stream", bufs=4) as spool,
            tc.tile_pool(name="opool", bufs=3) as opool,
            tc.tile_pool(name="ps_small", bufs=2, space="PSUM") as ps_small,
            tc.tile_pool(name="ps_r", bufs=1, space="PSUM") as ps_r,
            tc.tile_pool(name="ps_o", bufs=2, space="PSUM") as ps_o,
        ):
            # --- constants / small inputs (ACT ring, decoupled from bulk) ---
            ident = cpool.tile([128, 128], F32, tag="ident")
            make_identity(nc, ident)
            wm_sb = cpool.tile([128, 2 * C_OUT], F32R, tag="wm")
            for k2 in range(2):
                nc.scalar.dma_start(
                    out=wm_sb[:, k2 * C_OUT : (k2 + 1) * C_OUT],
                    in_=wm.ap()[k2 * 128 : (k2 + 1) * 128, :],
                )
            ft_sb = [
                cpool.tile([128, N], F32R, tag=f"ft{k2}", name=f"ft_sb{k2}")
                for k2 in range(2)
            ]
            for k2 in range(2):
                nc.scalar.dma_start(out=ft_sb[k2], in_=ft.ap()[k2 * 128 : (k2 + 1) * 128, :])

            # --- stage A: T = F @ W, natural k-tiles, 4 per [128,512] group ---
            t_sb = [
                cpool.tile([128, 4 * 128], F32R, tag=f"T{g}", name=f"t_sb{g}")
                for g in range(KT // 4)
            ]
            for g in range(KT // 4):
                ps = ps_small.tile([128, 512], F32, tag="psA")
                for i in range(4):
                    n_tile = g * 4 + i
                    for k2 in range(2):
                        nc.tensor.matmul(
                            ps[:, i * 128 : (i + 1) * 128],
                            ft_sb[k2][:, n_tile * 128 : (n_tile + 1) * 128],
                            wm_sb[:, k2 * C_OUT : (k2 + 1) * C_OUT],
                            start=(k2 == 0),
                            stop=(k2 == 1),
                        )
                nc.vector.tensor_copy(t_sb[g], ps)

            # --- stage B: SR^T accumulation over 64 k-tiles ---
            # 2 k-tiles (256 winv_t rows, 1MB contiguous) per DMA instruction;
            # alternate the two HWDGE rings so issue overhead overlaps.
            ps_sr = ps_r.tile([128, B], F32, tag="psR")
            for g in range(KT // 2):
                wi = spool.tile([128, 2 * B], F32R, tag="wi", bufs=4)
                src = winv_t.ap()[g * 256 : (g + 1) * 256, :].rearrange(
                    "(a p) f -> p a f", a=2
                )
                eng = nc.sync if g % 2 == 0 else nc.scalar
                eng.dma_start(out=wi.rearrange("p (a f) -> p a f", a=2), in_=src)
                for a in range(2):
                    k = 2 * g + a
                    lhs = t_sb[k // 4][:, (k % 4) * 128 : (k % 4 + 1) * 128]
                    for h in range(2):
                        nc.tensor.matmul(
                            ps_sr[:, h * 512 : (h + 1) * 512],
                            lhs,
                            wi[:, a * B + h * 512 : a * B + (h + 1) * 512],
                            start=(k == 0),
                            stop=(k == KT - 1),
                        )

            srT = cpool.tile([128, B], F32, tag="srT")
            nc.vector.tensor_copy(srT, ps_sr)

            # --- stage C: SR tiles = transpose(SR^T) ---
            sr_sb = [
                cpool.tile([128, 128], F32R, tag=f"sr{mt}", name=f"sr_sb{mt}")
                for mt in range(MT)
            ]
            for mt in range(MT):
                pst = ps_small.tile([128, 128], F32, tag="psA")
                nc.tensor.transpose(pst, srT[:, mt * 128 : (mt + 1) * 128], ident)
                nc.vector.tensor_copy(sr_sb[mt], pst)

            # --- stage D: out^T partial chunks ---
            for ncch in range(N // NCH):
                ps_out = ps_o.tile([128, NCH], F32, tag="psO")
                for mt in range(MT):
                    wv = spool.tile([128, NCH], F32R, tag="wv", bufs=6)
                    eng = nc.sync if (ncch * MT + mt) % 2 == 0 else nc.scalar
                    eng.dma_start(
                        out=wv,
                        in_=wav_t.ap()[
                            mt * 128 : (mt + 1) * 128, ncch * NCH : (ncch + 1) * NCH
                        ],
                    )
                    for h in range(NCH // 512):
                        nc.tensor.matmul(
                            ps_out[:, h * 512 : (h + 1) * 512],
                            sr_sb[mt],
                            wv[:, h * 512 : (h + 1) * 512],
                            start=(mt == 0),
                            stop=(mt == MT - 1),
                        )
                ot = opool.tile([128, NCH], F32, tag="ot")
                nc.vector.tensor_copy(ot, ps_out)
                nc.gpsimd.dma_start(
                    out=outp.ap()[:, ncch * NCH : (ncch + 1) * NCH], in_=ot
                )
    nc.compile()
    return nc


def kernel(features, wavelets, wavelets_inv, weight_matrix, filt):
    os.environ.setdefault("BASS_NEVER_TRACE", "1")
    if "nc" not in _cache:
        _cache["nc"] = _build()
    nc = _cache["nc"]

    features = np.ascontiguousarray(features, dtype=np.float32)
    wavelets = np.ascontiguousarray(wavelets, dtype=np.float32)
    wavelets_inv = np.ascontiguousarray(wavelets_inv, dtype=np.float32)
    weight_matrix = np.ascontiguousarray(weight_matrix, dtype=np.float32)
    filt = np.ascontiguousarray(filt, dtype=np.float32)

    ft = np.ascontiguousarray(features.T)
    in_maps = []
    for j in range(M):
        jb = slice(j * B, (j + 1) * B)
        winv_t = np.ascontiguousarray((wavelets_inv[jb, :] * filt[jb, None]).T)
        wav_t = np.ascontiguousarray(wavelets[:, jb].T)
        in_maps.append({"ft": ft, "wm": weight_matrix, "winv_t": winv_t, "wav_t": wav_t})

    res = run_bass_kernel_spmd(nc, in_maps, core_ids=list(range(M)))
    acc = res.results[0]["outp"].astype(np.float64)
    for j in range(1, M):
        acc += res.results[j]["outp"]
    return np.ascontiguousarray(acc.T.astype(np.float32))


# revision 18
# speedup vs baseline: 2.2274x; 1.7352x over previous
"""Graph Wavelet Neural Network forward pass on 8 Trainium2 NeuronCores.

Computation: out = wavelets @ diag(filt) @ wavelets_inv @ features @ W
  N=8192, C_IN=256, C_OUT=128.

Strategy (memory regime: streaming the two [8192,8192] matrices dominates):
  - Core j owns row-block jb of wavelets_inv (-> right rows jb) and
    column-block jb of wavelets (-> full-shape partial of out; host sums
    the 8 partials). No device collectives.
  - Operands are pre-transposed on the host so the contraction index lands
    on SBUF partitions and every DMA is contiguous:
      ft     = features.T                 [256, 8192]  (replicated)
      winv_t = (filt * wavelets_inv)[jb].T [8192, 1024] (per-core)
      wav_t  = wavelets[:, jb].T           [1024, 8192] (per-core)
    filt is folded into wavelets_inv rows on the host (free O(N^2)).
  - The big streams are downcast to bf16 on the host: halves HBM traffic
    (this kernel's roofline) and runs the PE at 1 cycle/row. PSUM
    accumulation stays fp32; the final output and host reduction are fp32.
  - Device pipeline (core j):
      T    = features @ W            PE form A, T k-tiles in SBUF
      SR^T = sum_k T[k].T @ winv_t[k]    [128, 1024] psum accumulation
      SR   = PE-transpose(SR^T)          8 tiles [128m, 128c]
      o^T  = sum_m SR[m].T @ wav_t[m, nch] per 1024-wide n-chunk
    Bulk DMAs alternate the two HWDGE rings (sync/scalar); output partials
    leave via the SWDGE (gpsimd) ring.
"""

import os

import numpy as np

import concourse.bass as bass
import concourse.mybir as mybir
import concourse.tile as tile
from concourse import bacc
from concourse.bass_utils import run_bass_kernel_spmd
from concourse.masks import make_identity

N = 8192
C_IN = 256
C_OUT = 128
M = 8  # cores
B = N // M  # 1024 rows per core
KT = N // 128  # 64 contraction tiles
MT = B // 128  # 8 row tiles per core block
NCH = 1024  # output free-dim chunk
F32 = mybir.dt.float32

STREAM = "bf16"  # "bf16" or "f32r" for the big streamed operands

_cache = {}


def _stream_dt():
    return mybir.dt.bfloat16 if STREAM == "bf16" else mybir.dt.float32r


def _stream_np():
    if STREAM == "bf16":
        import ml_dtypes

        return ml_dtypes.bfloat16
    return np.float32


def _build():
    SDT = _stream_dt()
    nc = bacc.Bacc("TRN2", target_bir_lowering=False, debug=False)
    ft = nc.dram_tensor("ft", [C_IN, N], SDT, kind="ExternalInput")
    wm = nc.dram_tensor("wm", [C_IN, C_OUT], SDT, kind="ExternalInput")
    winv_t = nc.dram_tensor("winv_t", [N, B], SDT, kind="ExternalInput")
    wav_t = nc.dram_tensor("wav_t", [B, N], SDT, kind="ExternalInput")
    outp = nc.dram_tensor("outp", [C_OUT, N], F32, kind="ExternalOutput")

    with tile.TileContext(nc) as tc:
        with (
            tc.tile_pool(name="const", bufs=1) as cpool,
            tc.tile_pool(name="stream", bufs=4) as spool,
            tc.tile_pool(name="opool", bufs=3) as opool,
            tc.tile_pool(name="ps_small", bufs=2, space="PSUM") as ps_small,
            tc.tile_pool(name="ps_r", bufs=1, space="PSUM") as ps_r,
            tc.tile_pool(name="ps_o", bufs=2, space="PSUM") as ps_o,
        ):
            # --- constants / small inputs ---
            ident = cpool.tile([128, 128], SDT, tag="ident")
            make_identity(nc, ident)
            wm_sb = cpool.tile([128, 2 * C_OUT], SDT, tag="wm")
            for k2 in range(2):
                nc.scalar.dma_start(
                    out=wm_sb[:, k2 * C_OUT : (k2 + 1) * C_OUT],
                    in_=wm.ap()[k2 * 128 : (k2 + 1) * 128, :],
                )
            ft_sb = [
                cpool.tile([128, N], SDT, tag=f"ft{k2}", name=f"ft_sb{k2}")
                for k2 in range(2)
            ]
            for k2 in range(2):
                nc.scalar.dma_start(out=ft_sb[k2], in_=ft.ap()[k2 * 128 : (k2 + 1) * 128, :])

            # --- stage A: T = F @ W, natural k-tiles, 4 per [128,512] group ---
            t_sb = [
                cpool.tile([128, 4 * 128], SDT, tag=f"T{g}", name=f"t_sb{g}")
                for g in range(KT // 4)
            ]
            for g in range(KT // 4):
                ps = ps_small.tile([128, 512], F32, tag="psA")
                for i in range(4):
                    n_tile = g * 4 + i
                    for k2 in range(2):
                        nc.tensor.matmul(
                            ps[:, i * 128 : (i + 1) * 128],
                            ft_sb[k2][:, n_tile * 128 : (n_tile + 1) * 128],
                            wm_sb[:, k2 * C_OUT : (k2 + 1) * C_OUT],
                            start=(k2 == 0),
                            stop=(k2 == 1),
                        )
                nc.vector.tensor_copy(t_sb[g], ps)

            # --- stage B: SR^T accumulation over 64 k-tiles ---
            # 2 k-tiles (256 winv_t rows, contiguous) per DMA instruction;
            # alternate the two HWDGE rings so issue overhead overlaps.
            ps_sr = ps_r.tile([128, B], F32, tag="psR")
            for g in range(KT // 2):
                wi = spool.tile([128, 2 * B], SDT, tag="wi", bufs=6)
                src = winv_t.ap()[g * 256 : (g + 1) * 256, :].rearrange(
                    "(a p) f -> p a f", a=2
                )
                eng = nc.sync if g % 2 == 0 else nc.scalar
                eng.dma_start(out=wi.rearrange("p (a f) -> p a f", a=2), in_=src)
                for a in range(2):
                    k = 2 * g + a
                    lhs = t_sb[k // 4][:, (k % 4) * 128 : (k % 4 + 1) * 128]
                    for h in range(2):
                        nc.tensor.matmul(
                            ps_sr[:, h * 512 : (h + 1) * 512],
                            lhs,
                            wi[:, a * B + h * 512 : a * B + (h + 1) * 512],
                            start=(k == 0),
                            stop=(k == KT - 1),
                        )

            srT = cpool.tile([128, B], SDT, tag="srT")
            nc.vector.tensor_copy(srT, ps_sr)

            # --- stage C: SR tiles = transpose(SR^T) ---
            sr_sb = [
                cpool.tile([128, 128], SDT, tag=f"sr{mt}", name=f"sr_sb{mt}")
                for mt in range(MT)
            ]
            for mt in range(MT):
                pst = ps_small.tile([128, 128], SDT, tag="psA")
                nc.tensor.transpose(pst, srT[:, mt * 128 : (mt + 1) * 128], ident)
                nc.vector.tensor_copy(sr_sb[mt], pst)

            # --- stage D: out^T partial chunks ---
            for ncch in range(N // NCH):
                ps_out = ps_o.tile([128, NCH], F32, tag="psO")
                for mt in range(MT):
                    wv = spool.tile([128, NCH], SDT, tag="wv", bufs=6)
                    eng = nc.sync if (ncch * MT + mt) % 2 == 0 else nc.scalar
                    eng.dma_start(
                        out=wv,
                        in_=wav_t.ap()[
                            mt * 128 : (mt + 1) * 128, ncch * NCH : (ncch + 1) * NCH
                        ],
                    )
                    for h in range(NCH // 512):
                        nc.tensor.matmul(
                            ps_out[:, h * 512 : (h + 1) * 512],
                            sr_sb[mt],
                            wv[:, h * 512 : (h + 1) * 512],
                            start=(mt == 0),
                            stop=(mt == MT - 1),
                        )
                ot = opool.tile([128, NCH], F32, tag="ot")
                nc.vector.tensor_copy(ot, ps_out)
                nc.gpsimd.dma_start(
                    out=outp.ap()[:, ncch * NCH : (ncch + 1) * NCH], in_=ot
                )
    nc.compile()
    return nc


def kernel(features, wavelets, wavelets_inv, weight_matrix, filt):
    os.environ.setdefault("BASS_NEVER_TRACE", "1")
    if "nc" not in _cache:
        _cache["nc"] = _build()
    nc = _cache["nc"]
    sdt = _stream_np()

    features = np.ascontiguousarray(features, dtype=np.float32)
    wavelets = np.ascontiguousarray(wavelets, dtype=np.float32)
    wavelets_inv = np.ascontiguousarray(wavelets_inv, dtype=np.float32)
    weight_matrix = np.ascontiguousarray(weight_matrix, dtype=np.float32)
    filt = np.ascontiguousarray(filt, dtype=np.float32)

    ft = np.ascontiguousarray(features.T).astype(sdt)
    wm = weight_matrix.astype(sdt)
    in_maps = []
    for j in range(M):
        jb = slice(j * B, (j + 1) * B)
        winv_t = np.ascontiguousarray((wavelets_inv[jb, :] * filt[jb, None]).T).astype(sdt)
        wav_t = np.ascontiguousarray(wavelets[:, jb].T).astype(sdt)
        in_maps.append({"ft": ft, "wm": wm, "winv_t": winv_t, "wav_t": wav_t})

    res = run_bass_kernel_spmd(nc, in_maps, core_ids=list(range(M)))
    acc = res.results[0]["outp"].astype(np.float64)
    for j in range(1, M):
        acc += res.results[j]["outp"]
    return np.ascontiguousarray(acc.T.astype(np.float32))


# revision 21
# speedup vs baseline: 2.5322x; 1.1368x over previous
"""Graph Wavelet Neural Network forward pass on 8 Trainium2 NeuronCores.

Computation: out = wavelets @ diag(filt) @ wavelets_inv @ features @ W
  N=8192, C_IN=256, C_OUT=128.

Strategy (memory regime: streaming the two [8192,8192] matrices dominates):
  - Core j owns row-block jb of wavelets_inv (-> right rows jb) and
    column-block jb of wavelets (-> full-shape partial of out; host sums
    the 8 partials). No device collectives.
  - Operands are pre-transposed on the host so the contraction index lands
    on SBUF partitions and every DMA is contiguous:
      ft     = features.T                 [256, 8192]  (replicated)
      winv_t = (filt * wavelets_inv)[jb].T [8192, 1024] (per-core)
      wav_t  = wavelets[:, jb].T           [1024, 8192] (per-core)
    filt is folded into wavelets_inv rows on the host (free O(N^2)).
  - The big streams are downcast to bf16 on the host: halves HBM traffic
    (this kernel's roofline) and runs the PE at 1 cycle/row. PSUM
    accumulation stays fp32; the final output and host reduction are fp32.
  - Device pipeline (core j):
      T    = features @ W            PE form A, T k-tiles in SBUF
      SR^T = sum_k T[k].T @ winv_t[k]    [128, 1024] psum accumulation
      SR   = PE-transpose(SR^T)          8 tiles [128m, 128c]
      o^T  = sum_m SR[m].T @ wav_t[m, nch] per 1024-wide n-chunk
    Bulk DMAs alternate the two HWDGE rings (sync/scalar); output partials
    leave via the SWDGE (gpsimd) ring.
"""

import os

import numpy as np

import concourse.bass as bass
import concourse.mybir as mybir
import concourse.tile as tile
from concourse import bacc
from concourse.bass_utils import run_bass_kernel_spmd
from concourse.masks import make_identity

N = 8192
C_IN = 256
C_OUT = 128
M = 8  # cores
B = N // M  # 1024 rows per core
KT = N // 128  # 64 contraction tiles
MT = B // 128  # 8 row tiles per core block
NCH = 1024  # output free-dim chunk
F32 = mybir.dt.float32

STREAM = "bf16"  # "bf16" or "f32r" for the big streamed operands

_cache = {}


def _stream_dt():
    return mybir.dt.bfloat16 if STREAM == "bf16" else mybir.dt.float32r


def _stream_np():
    if STREAM == "bf16":
        import ml_dtypes

        return ml_dtypes.bfloat16
    return np.float32


def _build():
    SDT = _stream_dt()
    nc = bacc.Bacc("TRN2", target_bir_lowering=False, debug=False)
    ft = nc.dram_tensor("ft", [C_IN, N], SDT, kind="ExternalInput")
    wm = nc.dram_tensor("wm", [C_IN, C_OUT], SDT, kind="ExternalInput")
    winv_t = nc.dram_tensor("winv_t", [N, B], SDT, kind="ExternalInput")
    wav_t = nc.dram_tensor("wav_t", [B, N], SDT, kind="ExternalInput")
    outp = nc.dram_tensor("outp", [C_OUT, N], F32, kind="ExternalOutput")

    with tile.TileContext(nc) as tc:
        with (
            tc.tile_pool(name="const", bufs=1) as cpool,
            tc.tile_pool(name="stream", bufs=4) as spool,
            tc.tile_pool(name="opool", bufs=3) as opool,
            tc.tile_pool(name="ps_small", bufs=2, space="PSUM") as ps_small,
            tc.tile_pool(name="ps_r", bufs=1, space="PSUM") as ps_r,
            tc.tile_pool(name="ps_o", bufs=2, space="PSUM") as ps_o,
        ):
            # --- constants / small inputs ---
            ident = cpool.tile([128, 128], SDT, tag="ident")
            make_identity(nc, ident)
            wm_sb = cpool.tile([128, 2 * C_OUT], SDT, tag="wm")
            for k2 in range(2):
                nc.scalar.dma_start(
                    out=wm_sb[:, k2 * C_OUT : (k2 + 1) * C_OUT],
                    in_=wm.ap()[k2 * 128 : (k2 + 1) * 128, :],
                )
            # ft split into quarter-column DMAs on both rings so stage A's
            # first groups unblock early instead of waiting for all 4MB.
            ft_sb = [
                cpool.tile([128, N], SDT, tag=f"ft{k2}", name=f"ft_sb{k2}")
                for k2 in range(2)
            ]
            for q in range(4):
                cols = slice(q * (N // 4), (q + 1) * (N // 4))
                for k2 in range(2):
                    eng = nc.sync if (2 * q + k2) % 2 == 0 else nc.scalar
                    eng.dma_start(
                        out=ft_sb[k2][:, cols],
                        in_=ft.ap()[k2 * 128 : (k2 + 1) * 128, cols],
                    )

            # --- stage A: T = F @ W, natural k-tiles, 4 per [128,512] group ---
            t_sb = [
                cpool.tile([128, 4 * 128], SDT, tag=f"T{g}", name=f"t_sb{g}")
                for g in range(KT // 4)
            ]
            for g in range(KT // 4):
                ps = ps_small.tile([128, 512], F32, tag="psA")
                for i in range(4):
                    n_tile = g * 4 + i
                    for k2 in range(2):
                        nc.tensor.matmul(
                            ps[:, i * 128 : (i + 1) * 128],
                            ft_sb[k2][:, n_tile * 128 : (n_tile + 1) * 128],
                            wm_sb[:, k2 * C_OUT : (k2 + 1) * C_OUT],
                            start=(k2 == 0),
                            stop=(k2 == 1),
                        )
                nc.vector.tensor_copy(t_sb[g], ps)

            # --- stage B: SR^T accumulation over 64 k-tiles ---
            # 2 k-tiles (256 winv_t rows, contiguous) per DMA instruction;
            # alternate the two HWDGE rings so issue overhead overlaps.
            ps_sr = ps_r.tile([128, B], F32, tag="psR")
            for g in range(KT // 2):
                wi = spool.tile([128, 2 * B], SDT, tag="wi", bufs=8)
                src = winv_t.ap()[g * 256 : (g + 1) * 256, :].rearrange(
                    "(a p) f -> p a f", a=2
                )
                eng = nc.sync if g % 2 == 0 else nc.scalar
                eng.dma_start(out=wi.rearrange("p (a f) -> p a f", a=2), in_=src)
                for a in range(2):
                    k = 2 * g + a
                    lhs = t_sb[k // 4][:, (k % 4) * 128 : (k % 4 + 1) * 128]
                    for h in range(2):
                        nc.tensor.matmul(
                            ps_sr[:, h * 512 : (h + 1) * 512],
                            lhs,
                            wi[:, a * B + h * 512 : a * B + (h + 1) * 512],
                            start=(k == 0),
                            stop=(k == KT - 1),
                        )

            srT = cpool.tile([128, B], SDT, tag="srT")
            nc.vector.tensor_copy(srT, ps_sr)

            # --- stage C: SR tiles = transpose(SR^T) ---
            sr_sb = [
                cpool.tile([128, 128], SDT, tag=f"sr{mt}", name=f"sr_sb{mt}")
                for mt in range(MT)
            ]
            for mt in range(MT):
                pst = ps_small.tile([128, 128], SDT, tag="psA")
                nc.tensor.transpose(pst, srT[:, mt * 128 : (mt + 1) * 128], ident)
                nc.vector.tensor_copy(sr_sb[mt], pst)

            # --- stage D: out^T partial chunks ---
            # 2 m-tiles (256 wav_t rows) per DMA instruction, as in stage B.
            for ncch in range(N // NCH):
                ps_out = ps_o.tile([128, NCH], F32, tag="psO")
                for mg in range(MT // 2):
                    wv = spool.tile([128, 2 * NCH], SDT, tag="wv", bufs=10)
                    src = wav_t.ap()[
                        mg * 256 : (mg + 1) * 256, ncch * NCH : (ncch + 1) * NCH
                    ].rearrange("(a p) f -> p a f", a=2)
                    eng = nc.sync if (ncch * (MT // 2) + mg) % 2 == 0 else nc.scalar
                    eng.dma_start(out=wv.rearrange("p (a f) -> p a f", a=2), in_=src)
                    for a in range(2):
                        mt = 2 * mg + a
                        for h in range(NCH // 512):
                            nc.tensor.matmul(
                                ps_out[:, h * 512 : (h + 1) * 512],
                                sr_sb[mt],
                                wv[:, a * NCH + h * 512 : a * NCH + (h + 1) * 512],
                                start=(mt == 0),
                                stop=(mt == MT - 1),
                            )
                ot = opool.tile([128, NCH], F32, tag="ot")
                nc.vector.tensor_copy(ot, ps_out)
                nc.gpsimd.dma_start(
                    out=outp.ap()[:, ncch * NCH : (ncch + 1) * NCH], in_=ot
                )
    nc.compile()
    return nc


def kernel(features, wavelets, wavelets_inv, weight_matrix, filt):
    os.environ.setdefault("BASS_NEVER_TRACE", "1")
    if "nc" not in _cache:
        _cache["nc"] = _build()
    nc = _cache["nc"]
    sdt = _stream_np()

    features = np.ascontiguousarray(features, dtype=np.float32)
    wavelets = np.ascontiguousarray(wavelets, dtype=np.float32)
    wavelets_inv = np.ascontiguousarray(wavelets_inv, dtype=np.float32)
    weight_matrix = np.ascontiguousarray(weight_matrix, dtype=np.float32)
    filt = np.ascontiguousarray(filt, dtype=np.float32)

    ft = np.ascontiguousarray(features.T).astype(sdt)
    wm = weight_matrix.astype(sdt)
    in_maps = []
    for j in range(M):
        jb = slice(j * B, (j + 1) * B)
        winv_t = np.ascontiguousarray((wavelets_inv[jb, :] * filt[jb, None]).T).astype(sdt)
        wav_t = np.ascontiguousarray(wavelets[:, jb].T).astype(sdt)
        in_maps.append({"ft": ft, "wm": wm, "winv_t": winv_t, "wav_t": wav_t})

    res = run_bass_kernel_spmd(nc, in_maps, core_ids=list(range(M)))
    acc = res.results[0]["outp"].astype(np.float64)
    for j in range(1, M):
        acc += res.results[j]["outp"]
    return np.ascontiguousarray(acc.T.astype(np.float32))


# revision 25
# speedup vs baseline: 2.6332x; 1.0399x over previous
"""Graph Wavelet Neural Network forward pass on 8 Trainium2 NeuronCores.

Computation: out = wavelets @ diag(filt) @ wavelets_inv @ features @ W
  N=8192, C_IN=256, C_OUT=128.

Strategy (memory regime: streaming the two [8192,8192] matrices dominates):
  - Core j owns row-block jb of wavelets_inv (-> right rows jb) and
    column-block jb of wavelets (-> full-shape partial of out; host sums
    the 8 partials). No device collectives.
  - Operands are pre-transposed on the host so the contraction index lands
    on SBUF partitions and every DMA is contiguous:
      ft     = features.T                 [256, 8192]  (replicated)
      winv_t = (filt * wavelets_inv)[jb].T [8192, 1024] (per-core)
      wav_t  = wavelets[:, jb].T           [1024, 8192] (per-core)
    filt is folded into wavelets_inv rows on the host (free O(N^2)).
  - The big streams are downcast to bf16 on the host: halves HBM traffic
    (this kernel's roofline) and runs the PE at 1 cycle/row. PSUM
    accumulation stays fp32; the final output and host reduction are fp32.
  - Device pipeline (core j):
      T    = features @ W            PE form A, T k-tiles in SBUF
      SR^T = sum_k T[k].T @ winv_t[k]    [128, 1024] psum accumulation
      SR   = PE-transpose(SR^T)          8 tiles [128m, 128c]
      o^T  = sum_m SR[m].T @ wav_t[m, nch] per 1024-wide n-chunk
    Bulk DMAs alternate the two HWDGE rings (sync/scalar); output partials
    leave via the SWDGE (gpsimd) ring.
"""

import os

import numpy as np

import concourse.bass as bass
import concourse.mybir as mybir
import concourse.tile as tile
from concourse import bacc
from concourse.bass_utils import run_bass_kernel_spmd
from concourse.masks import make_identity

N = 8192
C_IN = 256
C_OUT = 128
M = 8  # cores
B = N // M  # 1024 rows per core
KT = N // 128  # 64 contraction tiles
MT = B // 128  # 8 row tiles per core block
NCH = 1024  # output free-dim chunk
F32 = mybir.dt.float32

STREAM = "bf16"  # "bf16" or "f32r" for the big streamed operands

_cache = {}


def _stream_dt():
    return mybir.dt.bfloat16 if STREAM == "bf16" else mybir.dt.float32r


def _stream_np():
    if STREAM == "bf16":
        import ml_dtypes

        return ml_dtypes.bfloat16
    return np.float32


def _build():
    SDT = _stream_dt()
    nc = bacc.Bacc("TRN2", target_bir_lowering=False, debug=False)
    ft = nc.dram_tensor("ft", [C_IN, N], SDT, kind="ExternalInput")
    wm = nc.dram_tensor("wm", [C_IN, C_OUT], SDT, kind="ExternalInput")
    winv_t = nc.dram_tensor("winv_t", [N, B], SDT, kind="ExternalInput")
    wav_t = nc.dram_tensor("wav_t", [B, N], SDT, kind="ExternalInput")
    outp = nc.dram_tensor("outp", [C_OUT, N], mybir.dt.bfloat16 if STREAM == "bf16" else F32, kind="ExternalOutput")

    with tile.TileContext(nc) as tc:
        with (
            tc.tile_pool(name="const", bufs=1) as cpool,
            tc.tile_pool(name="stream", bufs=4) as spool,
            tc.tile_pool(name="opool", bufs=3) as opool,
            tc.tile_pool(name="ps_small", bufs=2, space="PSUM") as ps_small,
            tc.tile_pool(name="ps_r", bufs=1, space="PSUM") as ps_r,
            tc.tile_pool(name="ps_o", bufs=2, space="PSUM") as ps_o,
        ):
            # --- constants / small inputs ---
            ident = cpool.tile([128, 128], SDT, tag="ident")
            make_identity(nc, ident)
            wm_sb = cpool.tile([128, 2 * C_OUT], SDT, tag="wm")
            for k2 in range(2):
                nc.scalar.dma_start(
                    out=wm_sb[:, k2 * C_OUT : (k2 + 1) * C_OUT],
                    in_=wm.ap()[k2 * 128 : (k2 + 1) * 128, :],
                )
            # ft split into quarter-column DMAs on both rings so stage A's
            # first groups unblock early instead of waiting for all 4MB.
            ft_sb = [
                cpool.tile([128, N], SDT, tag=f"ft{k2}", name=f"ft_sb{k2}")
                for k2 in range(2)
            ]
            for q in range(4):
                cols = slice(q * (N // 4), (q + 1) * (N // 4))
                for k2 in range(2):
                    eng = nc.sync if (2 * q + k2) % 2 == 0 else nc.scalar
                    eng.dma_start(
                        out=ft_sb[k2][:, cols],
                        in_=ft.ap()[k2 * 128 : (k2 + 1) * 128, cols],
                    )

            # --- stages A+B interleaved in PE program order ---
            # A-group g produces T k-tiles [4g, 4g+4); the two stage-B wi
            # groups that consume exactly those tiles follow immediately, so
            # the PE never waits on distant ft DMAs and wi slots recycle fast.
            t_sb = [
                cpool.tile([128, 4 * 128], SDT, tag=f"T{g}", name=f"t_sb{g}")
                for g in range(KT // 4)
            ]
            ps_sr = ps_r.tile([128, B], F32, tag="psR")
            for g in range(KT // 4):
                ps = ps_small.tile([128, 512], F32, tag="psA")
                for i in range(4):
                    n_tile = g * 4 + i
                    for k2 in range(2):
                        nc.tensor.matmul(
                            ps[:, i * 128 : (i + 1) * 128],
                            ft_sb[k2][:, n_tile * 128 : (n_tile + 1) * 128],
                            wm_sb[:, k2 * C_OUT : (k2 + 1) * C_OUT],
                            start=(k2 == 0),
                            stop=(k2 == 1),
                        )
                nc.vector.tensor_copy(t_sb[g], ps)
                for gg in (2 * g, 2 * g + 1):  # wi groups over k=[4g, 4g+4)
                    wi = spool.tile([128, 2 * B], SDT, tag="wi", bufs=10)
                    src = winv_t.ap()[gg * 256 : (gg + 1) * 256, :].rearrange(
                        "(a p) f -> p a f", a=2
                    )
                    eng = nc.sync if gg % 2 == 0 else nc.scalar
                    eng.dma_start(out=wi.rearrange("p (a f) -> p a f", a=2), in_=src)
                    for a in range(2):
                        k = 2 * gg + a
                        lhs = t_sb[k // 4][:, (k % 4) * 128 : (k % 4 + 1) * 128]
                        for h in range(2):
                            nc.tensor.matmul(
                                ps_sr[:, h * 512 : (h + 1) * 512],
                                lhs,
                                wi[:, a * B + h * 512 : a * B + (h + 1) * 512],
                                start=(k == 0),
                                stop=(k == KT - 1),
                            )

            srT = cpool.tile([128, B], SDT, tag="srT")
            nc.vector.tensor_copy(srT, ps_sr)

            # --- stage C: SR tiles = transpose(SR^T) ---
            sr_sb = [
                cpool.tile([128, 128], SDT, tag=f"sr{mt}", name=f"sr_sb{mt}")
                for mt in range(MT)
            ]
            for mt in range(MT):
                pst = ps_small.tile([128, 128], SDT, tag="psA")
                nc.tensor.transpose(pst, srT[:, mt * 128 : (mt + 1) * 128], ident)
                nc.vector.tensor_copy(sr_sb[mt], pst)

            # --- stage D: out^T partial chunks ---
            # 2 m-tiles (256 wav_t rows) per DMA instruction, as in stage B.
            for ncch in range(N // NCH):
                ps_out = ps_o.tile([128, NCH], F32, tag="psO")
                for mg in range(MT // 2):
                    wv = spool.tile([128, 2 * NCH], SDT, tag="wv", bufs=12)
                    src = wav_t.ap()[
                        mg * 256 : (mg + 1) * 256, ncch * NCH : (ncch + 1) * NCH
                    ].rearrange("(a p) f -> p a f", a=2)
                    eng = nc.sync if (ncch * (MT // 2) + mg) % 2 == 0 else nc.scalar
                    eng.dma_start(out=wv.rearrange("p (a f) -> p a f", a=2), in_=src)
                    for a in range(2):
                        mt = 2 * mg + a
                        for h in range(NCH // 512):
                            nc.tensor.matmul(
                                ps_out[:, h * 512 : (h + 1) * 512],
                                sr_sb[mt],
                                wv[:, a * NCH + h * 512 : a * NCH + (h + 1) * 512],
                                start=(mt == 0),
                                stop=(mt == MT - 1),
                            )
                ot = opool.tile([128, NCH], SDT if STREAM == "bf16" else F32, tag="ot")
                nc.vector.tensor_copy(ot, ps_out)
                eng = nc.scalar if ncch % 2 == 0 else nc.sync
                eng.dma_start(
                    out=outp.ap()[:, ncch * NCH : (ncch + 1) * NCH], in_=ot
                )
    nc.compile()
    return nc


def kernel(features, wavelets, wavelets_inv, weight_matrix, filt):
    os.environ.setdefault("BASS_NEVER_TRACE", "1")
    if "nc" not in _cache:
        _cache["nc"] = _build()
    nc = _cache["nc"]
    sdt = _stream_np()

    features = np.ascontiguousarray(features, dtype=np.float32)
    wavelets = np.ascontiguousarray(wavelets, dtype=np.float32)
    wavelets_inv = np.ascontiguousarray(wavelets_inv, dtype=np.float32)
    weight_matrix = np.ascontiguousarray(weight_matrix, dtype=np.float32)
    filt = np.ascontiguousarray(filt, dtype=np.float32)

    ft = np.ascontiguousarray(features.T).astype(sdt)
    wm = weight_matrix.astype(sdt)
    in_maps = []
    for j in range(M):
        jb = slice(j * B, (j + 1) * B)
        winv_t = np.ascontiguousarray((wavelets_inv[jb, :] * filt[jb, None]).T).astype(sdt)
        wav_t = np.ascontiguousarray(wavelets[:, jb].T).astype(sdt)
        in_maps.append({"ft": ft, "wm": wm, "winv_t": winv_t, "wav_t": wav_t})

    res = run_bass_kernel_spmd(nc, in_maps, core_ids=list(range(M)))
    acc = res.results[0]["outp"].astype(np.float64)
    for j in range(1, M):
        acc += res.results[j]["outp"]
    return np.ascontiguousarray(acc.T.astype(np.float32))


# revision 27
# speedup vs baseline: 2.7205x; 1.0332x over previous
"""Graph Wavelet Neural Network forward pass on 8 Trainium2 NeuronCores.

Computation: out = wavelets @ diag(filt) @ wavelets_inv @ features @ W
  N=8192, C_IN=256, C_OUT=128.

Strategy (memory regime: streaming the two [8192,8192] matrices dominates):
  - Core j owns row-block jb of wavelets_inv (-> right rows jb) and
    column-block jb of wavelets (-> full-shape partial of out; host sums
    the 8 partials). No device collectives.
  - Operands are pre-transposed/pre-blocked on the host so the contraction
    index lands on SBUF partitions and EVERY device DMA is one fully
    contiguous block:
      ft     = features.T                  [256, 8192]   (replicated)
      winv_t = (filt * wavelets_inv)[jb].T  [8192, 1024]  (per-core)
      wav_b  = wavelets[:, jb].T chunk-major [8*1024, 1024] (per-core),
               row ncch*1024 + m holds wav_t[m, ncch*1024 : ...]
    filt is folded into wavelets_inv rows on the host (free O(N^2)).
  - Big streams are bf16: halves HBM traffic (the roofline) and runs the
    PE at 1 cycle/row. PSUM accumulation stays fp32. Output partials are
    bf16 (their host fp64 sum adds ~1e-4 relative error) and leave in a
    chunk-major [8*128, 1024] layout so writes are contiguous too.
  - Device pipeline (core j):
      T    = features @ W              PE form A, T k-tiles in SBUF
      SR^T = sum_k T[k].T @ winv_t[k]  [128, 1024] psum accumulation
      SR   = PE-transpose(SR^T)        8 tiles [128m, 128c]
      o^T  = sum_m SR[m].T @ wav[m, nch]  per 1024-wide n-chunk
    Stage A groups interleave with stage B consumers in PE program order.
    Bulk DMAs are 1MB contiguous, alternating the two HWDGE rings.
"""

import os

import numpy as np

import concourse.bass as bass
import concourse.mybir as mybir
import concourse.tile as tile
from concourse import bacc
from concourse.bass_utils import run_bass_kernel_spmd
from concourse.masks import make_identity

N = 8192
C_IN = 256
C_OUT = 128
M = 8  # cores
B = N // M  # 1024 rows per core
KT = N // 128  # 64 contraction tiles
MT = B // 128  # 8 row tiles per core block
NCH = 1024  # output free-dim chunk
NC = N // NCH  # 8 chunks
F32 = mybir.dt.float32

STREAM = "bf16"  # "bf16" or "f32r" for the big streamed operands

_cache = {}


def _stream_dt():
    return mybir.dt.bfloat16 if STREAM == "bf16" else mybir.dt.float32r


def _stream_np():
    if STREAM == "bf16":
        import ml_dtypes

        return ml_dtypes.bfloat16
    return np.float32


def _build():
    SDT = _stream_dt()
    nc = bacc.Bacc("TRN2", target_bir_lowering=False, debug=False)
    ft = nc.dram_tensor("ft", [C_IN, N], SDT, kind="ExternalInput")
    wm = nc.dram_tensor("wm", [C_IN, C_OUT], SDT, kind="ExternalInput")
    winv_t = nc.dram_tensor("winv_t", [N, B], SDT, kind="ExternalInput")
    wav_b = nc.dram_tensor("wav_b", [NC * B, NCH], SDT, kind="ExternalInput")
    outp = nc.dram_tensor("outp", [NC * C_OUT, NCH], SDT, kind="ExternalOutput")

    with tile.TileContext(nc) as tc:
        with (
            tc.tile_pool(name="const", bufs=1) as cpool,
            tc.tile_pool(name="stream", bufs=4) as spool,
            tc.tile_pool(name="opool", bufs=3) as opool,
            tc.tile_pool(name="ps_small", bufs=2, space="PSUM") as ps_small,
            tc.tile_pool(name="ps_r", bufs=1, space="PSUM") as ps_r,
            tc.tile_pool(name="ps_o", bufs=2, space="PSUM") as ps_o,
        ):
            # --- constants / small inputs ---
            ident = cpool.tile([128, 128], SDT, tag="ident")
            make_identity(nc, ident)
            wm_sb = cpool.tile([128, 2 * C_OUT], SDT, tag="wm")
            for k2 in range(2):
                nc.scalar.dma_start(
                    out=wm_sb[:, k2 * C_OUT : (k2 + 1) * C_OUT],
                    in_=wm.ap()[k2 * 128 : (k2 + 1) * 128, :],
                )
            # ft split into quarter-column DMAs on both rings so stage A's
            # first groups unblock early instead of waiting for all 4MB.
            ft_sb = [
                cpool.tile([128, N], SDT, tag=f"ft{k2}", name=f"ft_sb{k2}")
                for k2 in range(2)
            ]
            for q in range(4):
                cols = slice(q * (N // 4), (q + 1) * (N // 4))
                for k2 in range(2):
                    eng = nc.sync if (2 * q + k2) % 2 == 0 else nc.scalar
                    eng.dma_start(
                        out=ft_sb[k2][:, cols],
                        in_=ft.ap()[k2 * 128 : (k2 + 1) * 128, cols],
                    )

            # --- stages A+B interleaved in PE program order ---
            # A-group g produces T k-tiles [4g, 4g+4); the 1MB wi DMA that
            # covers exactly those four k-tiles follows immediately.
            t_sb = [
                cpool.tile([128, 4 * 128], SDT, tag=f"T{g}", name=f"t_sb{g}")
                for g in range(KT // 4)
            ]
            ps_sr = ps_r.tile([128, B], F32, tag="psR")
            for g in range(KT // 4):
                ps = ps_small.tile([128, 512], F32, tag="psA")
                for i in range(4):
                    n_tile = g * 4 + i
                    for k2 in range(2):
                        nc.tensor.matmul(
                            ps[:, i * 128 : (i + 1) * 128],
                            ft_sb[k2][:, n_tile * 128 : (n_tile + 1) * 128],
                            wm_sb[:, k2 * C_OUT : (k2 + 1) * C_OUT],
                            start=(k2 == 0),
                            stop=(k2 == 1),
                        )
                nc.vector.tensor_copy(t_sb[g], ps)
                wi = spool.tile([128, 4 * B], SDT, tag="wi", bufs=6)
                src = winv_t.ap()[g * 512 : (g + 1) * 512, :].rearrange(
                    "(a p) f -> p a f", a=4
                )
                eng = nc.sync if g % 2 == 0 else nc.scalar
                eng.dma_start(out=wi.rearrange("p (a f) -> p a f", a=4), in_=src)
                for a in range(4):
                    k = 4 * g + a
                    lhs = t_sb[g][:, a * 128 : (a + 1) * 128]
                    for h in range(2):
                        nc.tensor.matmul(
                            ps_sr[:, h * 512 : (h + 1) * 512],
                            lhs,
                            wi[:, a * B + h * 512 : a * B + (h + 1) * 512],
                            start=(k == 0),
                            stop=(k == KT - 1),
                        )

            srT = cpool.tile([128, B], SDT, tag="srT")
            nc.vector.tensor_copy(srT, ps_sr)

            # --- stage C: SR tiles = transpose(SR^T) ---
            sr_sb = [
                cpool.tile([128, 128], SDT, tag=f"sr{mt}", name=f"sr_sb{mt}")
                for mt in range(MT)
            ]
            for mt in range(MT):
                pst = ps_small.tile([128, 128], SDT, tag="psA")
                nc.tensor.transpose(pst, srT[:, mt * 128 : (mt + 1) * 128], ident)
                nc.vector.tensor_copy(sr_sb[mt], pst)

            # --- stage D: out^T partial chunks; all DMAs 1MB contiguous ---
            for ncch in range(NC):
                ps_out = ps_o.tile([128, NCH], F32, tag="psO")
                for mg in range(2):
                    wv = spool.tile([128, 4 * NCH], SDT, tag="wv", bufs=6)
                    src = wav_b.ap()[
                        ncch * B + mg * 512 : ncch * B + (mg + 1) * 512, :
                    ].rearrange("(a p) f -> p a f", a=4)
                    eng = nc.sync if (ncch * 2 + mg) % 2 == 0 else nc.scalar
                    eng.dma_start(out=wv.rearrange("p (a f) -> p a f", a=4), in_=src)
                    for a in range(4):
                        mt = 4 * mg + a
                        for h in range(NCH // 512):
                            nc.tensor.matmul(
                                ps_out[:, h * 512 : (h + 1) * 512],
                                sr_sb[mt],
                                wv[:, a * NCH + h * 512 : a * NCH + (h + 1) * 512],
                                start=(mt == 0),
                                stop=(mt == MT - 1),
                            )
                ot = opool.tile([128, NCH], SDT, tag="ot")
                nc.vector.tensor_copy(ot, ps_out)
                eng = nc.scalar if ncch % 2 == 0 else nc.sync
                eng.dma_start(
                    out=outp.ap()[ncch * C_OUT : (ncch + 1) * C_OUT, :], in_=ot
                )
    nc.compile()
    return nc


def make_in_maps(features, wavelets, wavelets_inv, weight_matrix, filt):
    sdt = _stream_np()
    features = np.ascontiguousarray(features, dtype=np.float32)
    wavelets = np.ascontiguousarray(wavelets, dtype=np.float32)
    wavelets_inv = np.ascontiguousarray(wavelets_inv, dtype=np.float32)
    weight_matrix = np.ascontiguousarray(weight_matrix, dtype=np.float32)
    filt = np.ascontiguousarray(filt, dtype=np.float32)

    ft = np.ascontiguousarray(features.T).astype(sdt)
    wm = weight_matrix.astype(sdt)
    in_maps = []
    for j in range(M):
        jb = slice(j * B, (j + 1) * B)
        winv_t = np.ascontiguousarray((wavelets_inv[jb, :] * filt[jb, None]).T).astype(sdt)
        # chunk-major blocking of wavelets[:, jb].T: row ncch*B + m
        wav_t = wavelets[:, jb].T  # [B, N]
        wav_b = np.ascontiguousarray(
            wav_t.reshape(B, NC, NCH).transpose(1, 0, 2).reshape(NC * B, NCH)
        ).astype(sdt)
        in_maps.append({"ft": ft, "wm": wm, "winv_t": winv_t, "wav_b": wav_b})
    return in_maps


def combine_outputs(results):
    acc = results[0]["outp"].astype(np.float64)
    for j in range(1, M):
        acc += results[j]["outp"]
    # outp rows are [ncch][c]: row ncch*C_OUT + c holds out^T[c, ncch*NCH:...]
    out_t = acc.reshape(NC, C_OUT, NCH).transpose(1, 0, 2).reshape(C_OUT, N)
    return np.ascontiguousarray(out_t.T.astype(np.float32))


def kernel(features, wavelets, wavelets_inv, weight_matrix, filt):
    os.environ.setdefault("BASS_NEVER_TRACE", "1")
    if "nc" not in _cache:
        _cache["nc"] = _build()
    nc = _cache["nc"]
    in_maps = make_in_maps(features, wavelets, wavelets_inv, weight_matrix, filt)
    res = run_bass_kernel_spmd(nc, in_maps, core_ids=list(range(M)))
    return combine_outputs(res.results)
